# revision 1
# baseline (speedup 1.0000x reference)
"""GAT kernel entry point (dev version; final must be self-contained)."""
import numpy as np

_LAST = None          # (SpmdKernel, in_maps) for test.py timing
_CACHE = {}


def kernel(x, edge_index, W, att_src, att_dst, bias, Wp, bp):
    global _LAST
    import gat_impl as gi
    from spmd_runner import SpmdKernel

    inputs = {"x": x, "edge_index": edge_index, "W": W, "att_src": att_src,
              "att_dst": att_dst, "bias": bias, "Wp": Wp, "bp": bp}
    prep = gi.host_prepare(x, edge_index, W, att_src, att_dst)

    key = ("gat", prep["ttot"], tuple(s[2] for s in prep["schedule"]))
    if key not in _CACHE:
        nc = gi.build_gat(prep["schedule"], prep["ttot"])
        nc.finalize()
        _CACHE[key] = SpmdKernel(nc, gi.CORES)
    k = _CACHE[key]

    in_maps = gi.make_in_maps(inputs, prep)
    results = k.run(in_maps)
    _LAST = (k, in_maps)
    out = gi.assemble_output(results)
    return out.astype(np.float32)
